# revision 1
# baseline (speedup 1.0000x reference)
"""Trainium2 Bass kernel for nn_GammaNeuronNet (conductance-based neuron network).

Strategy
--------
N=4096 neurons, 300 sequential timesteps. Per step, three matvecs against two
constant 4096x4096 matrices (G_syn used twice, G_gap once), then an
elementwise state update of (V, s).

* Row-partition G_syn/G_gap across the 8 cores (512 rows each). Both shards
  are cast to bf16 and kept SBUF-resident for the whole kernel (8 MB/core),
  so HBM is only touched once for the matrices.
* The two matrices are merged along the contraction axis: one accumulation
  of 64 k-tiles computes   col0 = G_syn @ s   (zeros for the G_gap half)
  and                      col1 = G_syn @ (s*E_syn) + G_gap @ V
  which is all the reference needs (int_syn and int_gap only appear summed).
* Matmuls are x-stationary: lhsT = 2 columns of the gathered x-tile, rhs =
  G^T tile [128,512] streamed, PSUM out [2,512]. PE-transposes convert
  [2,512] into the [128, 4] per-row layout used by the elementwise update.
* The elementwise update uses the identity
      V_inf - V = dV / denom   =>   vstep = dV * min(dt, 1/denom)
  which is mathematically identical to the reference's clip().
* Per step, each core computes the bf16 matmul operand values for its own
  512 neurons -- laid out exactly as the next step's stationary-weight tile
  rows -- and an 8-core AllGather concatenates them into the full [128,128]
  bf16 "xw" tile. The AllGather output is DMA'd straight into SBUF and used
  verbatim; no per-step relayout or rebuild is needed.

Global state layout ("L2"): neuron n maps to row n//32, sub-col n%32. The
exchanged tile xw[p, 32*g + t] holds quantity g of neuron k = 32p + t, with
quantities g = [zero, V, s, s*E_syn]. Matmul k-tile t uses lhsT columns
{64+t, 96+t} (s, sE) for G_syn and {t, 32+t} (0, V) for G_gap.
"""

import os
import numpy as np
import ml_dtypes

N = 4096
NCORES = 8
ROWS = N // NCORES            # 512 matrix rows per core
MT = ROWS // 128              # 4 m-tiles of 128 rows
KTM = N // 128                # 32 k-tiles per matrix
KT = 2 * KTM                  # 64 merged k-tiles (G_syn then G_gap)
BETA, V_TH, A_R, A_D = 0.125, -15.0, 1.0, 5.0

_cache = {}
last_results = None


def _n_steps(timestep, runtime):
    # replicate the reference's float-accumulation loop exactly
    t, n = 0.0, 0
    while t < runtime:
        t += timestep
        n += 1
    return n


def _build(n_steps: int, dt: float):
    import concourse.bacc as bacc
    import concourse.mybir as mybir
    import concourse.tile as tile
    from concourse import masks

    f32 = mybir.dt.float32
    bf16 = mybir.dt.bfloat16

    nc = bacc.Bacc("TRN2", target_bir_lowering=False, debug=False,
                   num_devices=NCORES)

    w_d = nc.dram_tensor("w_in", [128, KT * ROWS], bf16, kind="ExternalInput")
    xw0_d = nc.dram_tensor("xw0_in", [128, 128], bf16, kind="ExternalInput")
    vs0_d = nc.dram_tensor("vs0_in", [128, 3 * MT], f32, kind="ExternalInput")
    cgl_d = nc.dram_tensor("cgl_in", [128, 2 * MT], f32, kind="ExternalInput")
    esyn_d = nc.dram_tensor("esyn_in", [128, MT], f32, kind="ExternalInput")
    vout_d = nc.dram_tensor("v_out", [128, MT], f32, kind="ExternalOutput")

    rg = [list(range(NCORES))]
    Sigmoid = mybir.ActivationFunctionType.Sigmoid
    Copy = mybir.ActivationFunctionType.Copy

    ar_dt = float(A_R) * dt              # u = ar_dt * sigmoid(...)
    c1 = 1.0 - float(A_D) * dt           # s_new = s*(c1 - u) + u
    sig_scale = float(BETA)
    sig_bias = -float(BETA) * float(V_TH)

    with tile.TileContext(nc) as tc:
        with (
            tc.tile_pool(name="const", bufs=1) as constp,
            tc.tile_pool(name="wpool", bufs=1) as wp,
            tc.tile_pool(name="xwpool", bufs=2) as xwp,
            tc.tile_pool(name="vs", bufs=2) as vsp,
            tc.tile_pool(name="ew", bufs=2) as ewp,
            tc.tile_pool(name="csb", bufs=2) as csbp,
            tc.tile_pool(name="mm", bufs=2, space="PSUM") as mmp,
            tc.tile_pool(name="pe", bufs=2, space="PSUM") as pep,
            tc.tile_pool(name="ttp", bufs=2, space="PSUM") as ttp,
            tc.tile_pool(name="dram", bufs=2, space="DRAM") as dramp,
        ):
            w_sb = wp.tile([128, KT * ROWS], bf16)
            nc.sync.dma_start(w_sb[:], w_d[:])
            cgl_sb = constp.tile([128, 2 * MT], f32)
            nc.sync.dma_start(cgl_sb[:], cgl_d[:])
            esyn_sb = constp.tile([128, MT], f32)
            nc.sync.dma_start(esyn_sb[:], esyn_d[:])
            ident = constp.tile([128, 128], f32)
            masks.make_identity(nc, ident[:])
            sigb_sb = constp.tile([128, 1], f32)
            nc.vector.memset(sigb_sb[:], sig_bias)

            # double-buffered tiles reused across steps by parity
            xw_bufs = [xwp.tile([128, 128], bf16, tag="xw", name=f"xwb{j}")
                       for j in range(2)]
            nc.sync.dma_start(xw_bufs[0][:], xw0_d[:])
            ccin_bufs = [dramp.tile([16, 128], bf16, tag="ccin", name=f"ccinb{j}")
                         for j in range(2)]
            # zero the exchange buffers once: quadrant g=0 must stay zero
            zsrc = constp.tile([16, 128], bf16)
            nc.vector.memset(zsrc[:], 0.0)
            nc.sync.dma_start(ccin_bufs[0][:], zsrc[:])
            nc.sync.dma_start(ccin_bufs[1][:], zsrc[:])

            vs = vsp.tile([128, 3 * MT], f32, tag="vs")
            nc.sync.dma_start(vs[:], vs0_d[:])

            for i in range(n_steps):
                last = i == n_steps - 1
                xw = xw_bufs[i % 2]
                V = vs[:, 0:MT]
                S = vs[:, MT:2 * MT]

                # ---- ACT precomputation from V_old (overlaps the MM burst)
                sg = ewp.tile([128, MT], f32, tag="sg")
                u = ewp.tile([128, MT], f32, tag="u")
                w_ = ewp.tile([128, MT], f32, tag="w")
                nc.scalar.activation(sg[:], V, Sigmoid, bias=sigb_sb[:, 0:1],
                                     scale=sig_scale)
                nc.scalar.activation(u[:], sg[:], Copy, bias=0.0, scale=ar_dt)
                nc.scalar.activation(w_[:], u[:], Copy, bias=c1, scale=-1.0)

                # ---- matvecs: 64 accumulating matmuls, out [2, 512]
                mm = mmp.tile([2, ROWS], f32, tag="mm")
                xw_r = xw[:].rearrange("p (g t) -> p t g", g=4)
                for kt in range(KT):
                    if kt < KTM:
                        lhsT = xw_r[:, kt, 2:4]          # {s, sE}
                    else:
                        lhsT = xw_r[:, kt - KTM, 0:2]    # {0, V}
                    nc.tensor.matmul(
                        mm[:, :],
                        lhsT,
                        w_sb[:, kt * ROWS:(kt + 1) * ROWS],
                        start=(kt == 0),
                        stop=(kt == KT - 1),
                    )

                # ---- PSUM [2,512] -> SBUF, 4 PE-transposes -> [128, (mt,j)]
                cs_sb = csbp.tile([2, ROWS], f32, tag="cs")
                nc.vector.tensor_copy(cs_sb[:], mm[:])
                pe_ps = pep.tile([128, 2 * MT], f32, tag="pe")
                for mt in range(MT):
                    nc.tensor.transpose(
                        pe_ps[:, 2 * mt:2 * mt + 2],
                        cs_sb[:, mt * 128:(mt + 1) * 128],
                        ident[:2, :2],
                    )

                # ---- elementwise update: vs_new = [V', s', s'*E_syn]
                dn = ewp.tile([128, 2 * MT], f32, tag="dn")
                dv = ewp.tile([128, MT], f32, tag="dv")
                r = ewp.tile([128, MT], f32, tag="r")
                p2 = ewp.tile([128, MT], f32, tag="p2")
                vs_new = vsp.tile([128, 3 * MT], f32, tag="vs")

                nc.vector.tensor_add(dn[:], pe_ps[:], cgl_sb[:])
                dn3 = dn[:].rearrange("p (m j) -> p m j", j=2)
                den = dn3[:, :, 0]
                num = dn3[:, :, 1]
                nc.vector.tensor_mul(dv[:], V, den)
                nc.vector.tensor_sub(dv[:], num, dv[:])          # num - V*den
                nc.vector.reciprocal(r[:], den)
                nc.vector.tensor_scalar_min(r[:], r[:], dt)      # min(1/den, dt)
                nc.vector.tensor_mul(dv[:], dv[:], r[:])         # vstep
                nc.vector.tensor_add(vs_new[:, 0:MT], V, dv[:])
                # s-chain (no matvec dependency -- the scheduler runs these
                # early, during the MM burst)
                nc.vector.tensor_mul(p2[:], S, w_[:])            # s*(c1-u)
                nc.vector.tensor_add(vs_new[:, MT:2 * MT], p2[:], u[:])
                nc.vector.tensor_mul(vs_new[:, 2 * MT:3 * MT],
                                     vs_new[:, MT:2 * MT], esyn_sb[:])

                vs = vs_new
                if last:
                    nc.sync.dma_start(vout_d[:], vs_new[:, 0:MT])
                    break

                # ---- exchange: transpose [128,12] -> [12,128], cast to bf16,
                #      one DMA into ccin quadrants [V|s|sE], AllGather -> next xw
                tt_ps = ttp.tile([3 * MT, 128], f32, tag="tt")
                nc.tensor.transpose(tt_ps[:], vs_new[:], ident[:128, :128])
                tt_sb = csbp.tile([3 * MT, 128], bf16, tag="ttsb")
                nc.vector.tensor_copy(tt_sb[:], tt_ps[:])

                ccin = ccin_bufs[i % 2]
                ccout = nc.dram_tensor(f"ccout{i}", [128, 128], bf16,
                                       addr_space="Shared")
                cc4 = ccin[:].rearrange("(r b) (g t) -> g r b t", b=4, g=4)
                for g, eng in ((0, nc.sync), (1, nc.scalar), (2, nc.gpsimd)):
                    eng.dma_start(
                        cc4[g + 1, :, :, :],
                        tt_sb[4 * g:4 * (g + 1), :].rearrange(
                            "r (b t) -> r b t", t=32),
                    )
                nc.gpsimd.collective_compute(
                    "AllGather",
                    mybir.AluOpType.bypass,
                    replica_groups=rg,
                    ins=[ccin[:].opt()],
                    outs=[ccout[:].opt()],
                )
                nc.sync.dma_start(xw_bufs[(i + 1) % 2][:], ccout[:])

    nc.compile()
    return nc


def _prep(input_V, G_leak, E_leak, G_syn, E_syn, G_gap):
    iv = np.asarray(input_V, np.float32).reshape(-1)
    G_leak = np.asarray(G_leak, np.float32)
    E_leak = np.asarray(E_leak, np.float32)
    G_syn = np.asarray(G_syn, np.float32)
    E_syn = np.asarray(E_syn, np.float32)
    G_gap = np.asarray(G_gap, np.float32)
    in_len = iv.shape[0]

    in_avg = np.float32(iv.mean(dtype=np.float32))
    V0 = np.concatenate([iv, np.full(N - in_len, in_avg, np.float32)])
    x = (BETA * (V0 - V_TH)).astype(np.float32)
    sig = (1.0 / (1.0 + np.exp(-x, dtype=np.float32))).astype(np.float32)
    s0 = (A_R * sig / (A_R * sig + A_D)).astype(np.float32)
    sE0 = (s0 * E_syn).astype(np.float32)
    co_gap = G_gap.sum(axis=1, dtype=np.float32)
    c0_full = (G_leak + co_gap).astype(np.float32)
    gle_full = (G_leak * E_leak).astype(np.float32)

    Gs16 = G_syn.astype(ml_dtypes.bfloat16)
    Gg16 = G_gap.astype(ml_dtypes.bfloat16)

    # initial stationary tile: [Z | V | s | sE], col 32g+t = quantity g of
    # neuron 32p+t
    xw0 = np.zeros((128, 4, 32), ml_dtypes.bfloat16)
    xw0[:, 1, :] = V0.reshape(128, 32)
    xw0[:, 2, :] = s0.reshape(128, 32)
    xw0[:, 3, :] = sE0.reshape(128, 32)
    xw0 = np.ascontiguousarray(xw0.reshape(128, 128))

    def pmlayout(v):
        # [512] per-core slice -> [128, MT] psum-layout
        return np.ascontiguousarray(v.reshape(MT, 128).T)

    in_maps = []
    for c in range(NCORES):
        rows = slice(c * ROWS, (c + 1) * ROWS)
        A_s = Gs16[rows, :].reshape(ROWS, 128, 32)   # [n, p, t], k = 32p + t
        A_g = Gg16[rows, :].reshape(ROWS, 128, 32)
        Ws = np.transpose(A_s, (1, 2, 0))            # [p, t, n]
        Wg = np.transpose(A_g, (1, 2, 0))
        W = np.ascontiguousarray(
            np.concatenate([Ws, Wg], axis=1)
        ).reshape(128, KT * ROWS)
        vs0 = np.concatenate(
            [pmlayout(V0[rows]), pmlayout(s0[rows]), pmlayout(sE0[rows])], axis=1
        )
        cgl = np.empty((128, 2 * MT), np.float32)
        cgl[:, 0::2] = pmlayout(c0_full[rows])
        cgl[:, 1::2] = pmlayout(gle_full[rows])
        in_maps.append({
            "w_in": W,
            "xw0_in": xw0,
            "vs0_in": np.ascontiguousarray(vs0),
            "cgl_in": np.ascontiguousarray(cgl),
            "esyn_in": pmlayout(E_syn[rows]),
        })
    return in_maps, in_len


def kernel(input_V, G_leak, E_leak, G_syn, E_syn, G_gap, timestep, runtime):
    global last_results
    from concourse.bass_utils import run_bass_kernel_spmd

    dt = float(np.asarray(timestep))
    rt = float(np.asarray(runtime))
    n_steps = _n_steps(dt, rt)

    key = (n_steps, dt)
    if key not in _cache:
        _cache[key] = _build(n_steps, dt)
    nc = _cache[key]

    in_maps, in_len = _prep(input_V, G_leak, E_leak, G_syn, E_syn, G_gap)
    trace = os.environ.get("GAMMA_TRACE", "0") == "1"
    res = run_bass_kernel_spmd(
        nc, in_maps, core_ids=list(range(NCORES)), trace=trace
    )
    last_results = res

    V = np.concatenate(
        [np.asarray(res.results[c]["v_out"]).T.reshape(ROWS) for c in range(NCORES)]
    ).astype(np.float32)
    V[in_len:] = 0.0
    return V



# revision 9
# speedup vs baseline: 1.3641x; 1.3641x over previous
"""Trainium2 Bass kernel for nn_GammaNeuronNet (conductance-based neuron network).

Strategy (v2)
-------------
N=4096 neurons, 300 sequential timesteps. Per step, three matvecs against two
constant 4096x4096 matrices (G_syn used twice for s and s*E_syn, G_gap once
for V), then an elementwise state update of (V, s).

* Row-partition G_syn/G_gap across the 8 cores (512 rows each). Both shards
  stay SBUF-resident for the whole kernel (fp8 or bf16).
* x-stationary matmuls: lhsT = 2 columns of per-neuron state, rhs = G^T tile
  streamed, PSUM out [2,512] accumulated over 64 merged k-tiles
  (col0 = co_syn, col1 = int_syn + int_gap).  With fp8, DoubleRow perf mode
  processes two k-tiles per instruction (2x rate); the fp8 scale factor S on
  G is folded into the precomputed constants (cgl * S, dt/S) so no descale
  ops are needed.
* The s-part of the state does NOT depend on the matvecs, so every core
  redundantly computes the FULL s / s*E_syn vectors locally each step.  Only
  V needs to be exchanged: one [4,128] bf16 DMA -> 8-core AllGather ->
  [32,128] -> one gather-DMA into the per-neuron layout.
* k-tile order puts all G_syn tiles (which need only the locally computed
  s/sE) first, G_gap (which needs the gathered V) last, so the V AllGather
  of step i hides behind the G_syn half of step i+1's matmul burst.

Layouts: "xw layout" puts neuron n at [n//32 (partition), n%32]; SSE/ZV
interleave pairs so k-tile t's lhsT is cols [2t, 2t+2).  The own-slice V
kept for the elementwise update uses "psum layout": local neuron l of the
core's 512 rows at [l%128, l//128], matching the transposed matmul output.
"""

import os
import numpy as np
import ml_dtypes

N = 4096
NCORES = 8
ROWS = N // NCORES            # 512 matrix rows per core
MT = ROWS // 128              # 4 m-tiles of 128 rows
KTM = N // 128                # 32 k-tiles per matrix
KT = 2 * KTM                  # 64 merged k-tiles (G_syn then G_gap)
BETA, V_TH, A_R, A_D = 0.125, -15.0, 1.0, 5.0

USE_FP8 = os.environ.get("GAMMA_FP8", "1") == "1"
FP8_SCALE = 2.0 ** 17         # G values <= 1e-3 -> scaled <= ~131 (fp8e4 max 240)

_cache = {}
last_results = None


def _n_steps(timestep, runtime):
    # replicate the reference's float-accumulation loop exactly
    t, n = 0.0, 0
    while t < runtime:
        t += timestep
        n += 1
    return n


def _build(n_steps: int, dt: float, use_fp8: bool):
    import concourse.bacc as bacc
    import concourse.mybir as mybir
    import concourse.tile as tile
    from concourse import masks

    f32 = mybir.dt.float32
    bf16 = mybir.dt.bfloat16
    xdt = mybir.dt.float8e4 if use_fp8 else bf16
    S = FP8_SCALE if use_fp8 else 1.0

    nc = bacc.Bacc("TRN2", target_bir_lowering=False, debug=False,
                   num_devices=NCORES)

    w_d = nc.dram_tensor("w_in", [128, KT * ROWS], xdt, kind="ExternalInput")
    sse0_d = nc.dram_tensor("sse0_in", [128, 64], xdt, kind="ExternalInput")
    zv0_d = nc.dram_tensor("zv0_in", [128, 64], xdt, kind="ExternalInput")
    vg0_d = nc.dram_tensor("vg0_in", [128, 32], bf16, kind="ExternalInput")
    sf0_d = nc.dram_tensor("sf0_in", [128, 32], f32, kind="ExternalInput")
    vs0_d = nc.dram_tensor("vs0_in", [128, MT], f32, kind="ExternalInput")
    cgl_d = nc.dram_tensor("cgl_in", [128, 2 * MT], f32, kind="ExternalInput")
    esyn_d = nc.dram_tensor("esyn_in", [128, 32], f32, kind="ExternalInput")
    vout_d = nc.dram_tensor("v_out", [128, MT], f32, kind="ExternalOutput")

    rg = [list(range(NCORES))]
    Sigmoid = mybir.ActivationFunctionType.Sigmoid
    Copy = mybir.ActivationFunctionType.Copy
    DR = mybir.MatmulPerfMode.DoubleRow
    Alu = mybir.AluOpType

    ar_dt = float(A_R) * dt              # u = ar_dt * sigmoid(...)
    c1 = 1.0 - float(A_D) * dt           # s_new = s*(c1 - u) + u
    sig_scale = float(BETA)
    sig_bias = -float(BETA) * float(V_TH)
    dtS = dt / S                         # vstep = dv_s * min(dt/S, 1/den_s)

    with tile.TileContext(nc) as tc:
        with (
            tc.tile_pool(name="const", bufs=1) as constp,
            tc.tile_pool(name="wpool", bufs=1) as wp,
            tc.tile_pool(name="state", bufs=1) as stp,
            tc.tile_pool(name="ew", bufs=2) as ewp,
            tc.tile_pool(name="mm", bufs=1, space="PSUM") as mmp,
            tc.tile_pool(name="pe", bufs=1, space="PSUM") as pep,
            tc.tile_pool(name="ttp", bufs=1, space="PSUM") as ttp,
            tc.tile_pool(name="dram", bufs=2, space="DRAM") as dramp,
        ):
            w_sb = wp.tile([128, KT * ROWS], xdt)
            nc.sync.dma_start(w_sb[:], w_d[:])
            cgl_sb = constp.tile([128, 2 * MT], f32)
            nc.sync.dma_start(cgl_sb[:], cgl_d[:])
            esyn_sb = constp.tile([128, 32], f32)
            nc.sync.dma_start(esyn_sb[:], esyn_d[:])
            ident = constp.tile([128, 128], f32)
            masks.make_identity(nc, ident[:])
            sigb_sb = constp.tile([128, 1], f32)
            nc.vector.memset(sigb_sb[:], sig_bias)

            # persistent double-buffered state tiles (index = step parity)
            SSE = [stp.tile([128, 64], xdt, name=f"sse{j}") for j in range(2)]
            ZV = [stp.tile([128, 64], xdt, name=f"zv{j}") for j in range(2)]
            Vg = [stp.tile([128, 32], bf16, name=f"vg{j}") for j in range(2)]
            sf = [stp.tile([128, 32], f32, name=f"sf{j}") for j in range(2)]
            vs = [stp.tile([128, MT], f32, name=f"vs{j}") for j in range(2)]
            nc.sync.dma_start(SSE[0][:], sse0_d[:])
            nc.sync.dma_start(ZV[0][:], zv0_d[:])
            nc.sync.dma_start(ZV[1][:], zv0_d[:])   # for the zero columns
            nc.sync.dma_start(Vg[0][:], vg0_d[:])
            nc.sync.dma_start(sf[0][:], sf0_d[:])
            nc.sync.dma_start(vs[0][:], vs0_d[:])

            ccin = [dramp.tile([MT, 128], bf16, tag="ccin", name=f"ccin{j}")
                    for j in range(2)]

            mm_ps = [mmp.tile([2, ROWS], f32, name=f"mm{j}") for j in range(2)]
            pe_ps = [pep.tile([128, 2 * MT], f32, name=f"pe{j}")
                     for j in range(2)]
            tt_ps = [ttp.tile([MT, 128], f32, name=f"tt{j}") for j in range(2)]

            for i in range(n_steps):
                q, nq = i % 2, (i + 1) % 2
                last = i == n_steps - 1
                mm = mm_ps[q]

                # ---- matmul burst: G_syn k-tiles first (local s/sE), then
                #      G_gap (needs the gathered V of this step)
                if use_fp8:
                    for j in range(KTM):          # pair j covers k-tiles 2j,2j+1
                        lhs = SSE[q] if j < KTM // 2 else ZV[q]
                        jj = j % (KTM // 2)
                        nc.tensor.matmul(
                            mm[:],
                            lhs[:, 4 * jj:4 * jj + 4].rearrange(
                                "p (i m) -> p i m", i=2),
                            w_sb[:, j * 1024:(j + 1) * 1024].rearrange(
                                "p (i n) -> p i n", i=2),
                            start=(j == 0),
                            stop=(j == KTM - 1),
                            perf_mode=DR,
                        )
                else:
                    for kt in range(KT):
                        lhs = SSE[q] if kt < KTM else ZV[q]
                        t = kt % KTM
                        nc.tensor.matmul(
                            mm[:],
                            lhs[:, 2 * t:2 * t + 2],
                            w_sb[:, kt * ROWS:(kt + 1) * ROWS],
                            start=(kt == 0),
                            stop=(kt == KT - 1),
                        )

                # ---- full-vector s-chain for step i+1 (scalar + gpsimd;
                #      overlaps the matmul burst; only needs Vg[q], sf[q])
                if not last:
                    sg = ewp.tile([128, 32], f32, tag="sg")
                    u = ewp.tile([128, 32], f32, tag="u")
                    w_ = ewp.tile([128, 32], f32, tag="w")
                    p2 = ewp.tile([128, 32], f32, tag="p2")
                    se = ewp.tile([128, 32], f32, tag="se")
                    nc.scalar.activation(sg[:], Vg[q][:], Sigmoid,
                                         bias=sigb_sb[:, 0:1], scale=sig_scale)
                    nc.scalar.activation(u[:], sg[:], Copy, bias=0.0,
                                         scale=ar_dt)
                    nc.scalar.activation(w_[:], u[:], Copy, bias=c1,
                                         scale=-1.0)
                    nc.vector.tensor_mul(p2[:], sf[q][:], w_[:])
                    nc.vector.tensor_add(sf[nq][:], p2[:], u[:])
                    nc.vector.tensor_mul(se[:], sf[nq][:], esyn_sb[:])
                    sse_n = SSE[nq][:].rearrange("p (t u) -> p t u", u=2)
                    nc.gpsimd.tensor_copy(sse_n[:, :, 0], sf[nq][:])
                    nc.gpsimd.tensor_copy(sse_n[:, :, 1], se[:])

                # ---- PSUM [2,512] -> SBUF in 4 chunks on 2 engines
                # (gpsimd has no PSUM port; scalar does)
                cs = ewp.tile([2, ROWS], f32, tag="cs")
                for mt in range(MT):
                    if mt < 2:
                        nc.vector.tensor_copy(
                            cs[:, mt * 128:(mt + 1) * 128],
                            mm[:, mt * 128:(mt + 1) * 128])
                    else:
                        nc.scalar.activation(
                            cs[:, mt * 128:(mt + 1) * 128],
                            mm[:, mt * 128:(mt + 1) * 128],
                            Copy, bias=0.0, scale=1.0)
                pe = pe_ps[q]
                for mt in range(MT):
                    nc.tensor.transpose(
                        pe[:, 2 * mt:2 * mt + 2],
                        cs[:, mt * 128:(mt + 1) * 128],
                        ident[:2, :2],
                    )

                # ---- elementwise V update (own 512 neurons, psum layout)
                dn = ewp.tile([128, 2 * MT], f32, tag="dn")
                t1 = ewp.tile([128, MT], f32, tag="t1")
                dv = ewp.tile([128, MT], f32, tag="dv")
                r = ewp.tile([128, MT], f32, tag="r")
                nc.vector.tensor_add(dn[:], pe[:], cgl_sb[:])
                dn3 = dn[:].rearrange("p (m j) -> p m j", j=2)
                den = dn3[:, :, 0]
                num = dn3[:, :, 1]
                nc.vector.reciprocal(r[:], den)
                nc.vector.tensor_mul(t1[:], vs[q][:], den)
                nc.vector.tensor_sub(dv[:], num, t1[:])
                # vstep = (min(1/den, dt/S)) * dv
                nc.vector.scalar_tensor_tensor(
                    t1[:], r[:], dtS, dv[:], op0=Alu.min, op1=Alu.mult)
                nc.vector.tensor_add(vs[nq][:], vs[q][:], t1[:])

                if last:
                    nc.sync.dma_start(vout_d[:], vs[nq][:])
                    break

                # ---- V exchange: transpose [128,MT] -> [MT,128], cast bf16,
                #      DMA out, AllGather, gather-DMA into Vg + fp8 cast to ZV
                tt = tt_ps[q]
                nc.tensor.transpose(tt[:], vs[nq][:], ident[:128, :128])
                tt_bf = ewp.tile([MT, 128], bf16, tag="ttbf")
                nc.vector.tensor_copy(tt_bf[:], tt[:])
                nc.sync.dma_start(ccin[nq][:], tt_bf[:])

                # gathered flat order: core c's [4,128] block = neurons
                # [512c, 512c+512) in local (mt, pi) order = flat index
                # 32p + t of the xw layout, so [128, 32] row-major IS Vg.
                ccout = nc.dram_tensor(f"ccout{i}", [128, 32], bf16,
                                       addr_space="Shared")
                nc.gpsimd.collective_compute(
                    "AllGather",
                    mybir.AluOpType.bypass,
                    replica_groups=rg,
                    ins=[ccin[nq][:].opt()],
                    outs=[ccout[:].opt()],
                )
                nc.sync.dma_start(Vg[nq][:], ccout[:])
                zv_n = ZV[nq][:].rearrange("p (t u) -> p t u", u=2)
                nc.gpsimd.tensor_copy(zv_n[:, :, 1], Vg[nq][:])

    nc.compile()
    return nc


def _prep(input_V, G_leak, E_leak, G_syn, E_syn, G_gap, use_fp8):
    iv = np.asarray(input_V, np.float32).reshape(-1)
    G_leak = np.asarray(G_leak, np.float32)
    E_leak = np.asarray(E_leak, np.float32)
    G_syn = np.asarray(G_syn, np.float32)
    E_syn = np.asarray(E_syn, np.float32)
    G_gap = np.asarray(G_gap, np.float32)
    in_len = iv.shape[0]
    S = np.float32(FP8_SCALE if use_fp8 else 1.0)
    wt = ml_dtypes.float8_e4m3fn if use_fp8 else ml_dtypes.bfloat16

    in_avg = np.float32(iv.mean(dtype=np.float32))
    V0 = np.concatenate([iv, np.full(N - in_len, in_avg, np.float32)])
    x = (BETA * (V0 - V_TH)).astype(np.float32)
    sig = (1.0 / (1.0 + np.exp(-x, dtype=np.float32))).astype(np.float32)
    s0 = (A_R * sig / (A_R * sig + A_D)).astype(np.float32)
    sE0 = (s0 * E_syn).astype(np.float32)
    co_gap = G_gap.sum(axis=1, dtype=np.float32)
    c0_full = ((G_leak + co_gap) * S).astype(np.float32)
    gle_full = (G_leak * E_leak * S).astype(np.float32)

    def xwl(v):
        # [N] full vector -> [128, 32] xw layout (neuron n at [n//32, n%32])
        return np.ascontiguousarray(v.reshape(128, 32))

    def pmlayout(v):
        # [512] per-core slice -> [128, MT] psum layout
        return np.ascontiguousarray(v.reshape(MT, 128).T)

    sse0 = np.zeros((128, 32, 2), np.float32)
    sse0[:, :, 0] = xwl(s0)
    sse0[:, :, 1] = xwl(sE0)
    sse0 = np.ascontiguousarray(sse0.reshape(128, 64).astype(wt))
    zv0 = np.zeros((128, 32, 2), np.float32)
    zv0[:, :, 1] = xwl(V0)
    zv0 = np.ascontiguousarray(
        zv0.reshape(128, 64).astype(ml_dtypes.bfloat16).astype(wt))
    vg0 = xwl(V0).astype(ml_dtypes.bfloat16)
    sf0 = xwl(s0)
    esyn_full = xwl(E_syn)

    Gs = (G_syn * S).astype(wt)
    Gg = (G_gap * S).astype(wt)

    in_maps = []
    for c in range(NCORES):
        rows = slice(c * ROWS, (c + 1) * ROWS)
        A_s = Gs[rows, :].reshape(ROWS, 128, 32)   # [n, p, t], k = 32p + t
        A_g = Gg[rows, :].reshape(ROWS, 128, 32)
        Ws = np.transpose(A_s, (1, 2, 0))          # [p, t, n]
        Wg = np.transpose(A_g, (1, 2, 0))
        W = np.ascontiguousarray(
            np.concatenate([Ws, Wg], axis=1)
        ).reshape(128, KT * ROWS)
        cgl = np.empty((128, 2 * MT), np.float32)
        cgl[:, 0::2] = pmlayout(c0_full[rows])
        cgl[:, 1::2] = pmlayout(gle_full[rows])
        in_maps.append({
            "w_in": W,
            "sse0_in": sse0,
            "zv0_in": zv0,
            "vg0_in": vg0,
            "sf0_in": sf0,
            "vs0_in": pmlayout(V0[rows]),
            "cgl_in": np.ascontiguousarray(cgl),
            "esyn_in": esyn_full,
        })
    return in_maps, in_len


def kernel(input_V, G_leak, E_leak, G_syn, E_syn, G_gap, timestep, runtime):
    global last_results
    from concourse.bass_utils import run_bass_kernel_spmd

    dt = float(np.asarray(timestep))
    rt = float(np.asarray(runtime))
    n_steps = _n_steps(dt, rt)

    key = (n_steps, dt, USE_FP8)
    if key not in _cache:
        _cache[key] = _build(n_steps, dt, USE_FP8)
    nc = _cache[key]

    in_maps, in_len = _prep(input_V, G_leak, E_leak, G_syn, E_syn, G_gap,
                            USE_FP8)
    trace = os.environ.get("GAMMA_TRACE", "0") == "1"
    res = run_bass_kernel_spmd(
        nc, in_maps, core_ids=list(range(NCORES)), trace=trace
    )
    last_results = res

    V = np.concatenate(
        [np.asarray(res.results[c]["v_out"]).T.reshape(ROWS)
         for c in range(NCORES)]
    ).astype(np.float32)
    V[in_len:] = 0.0
    return V
